# revision 43
# baseline (speedup 1.0000x reference)
"""Trainium2 Bass kernel for nn_Dwtpool (dense_cnn).

Reference graph (per image, C=256, 128x128 input):
  p    = maxpool2x2(x)                          -> [256, 64, 64]
  r    = ReLU(BN(conv1x1(x, reduce_w)))         -> [ 64,128,128]
  M    = haar_dwt(r) * 2  (stored unscaled)     -> [256, 64, 64]
  q2..q4 = conv{3,5,7}(0.5*M)                   -> [256, 64, 64] each
  qkv  = conv3x3(concat[0.5*M, q1..q4, p])      -> [256, 64, 64]
  att  = softmax_spatial(conv1x1(qkv)); pooled = sum_n ch(qkv)_c,n * att_n
  cw   = ct2(ReLU(LN(ct1(pooled))))             -> [256]
  out  = conv1x1(qkv * cw, proj_w)              -> [256, 64, 64]

Strategy: data-parallel over batch (16 images / 8 cores = 2 per core), fp16
trunk.  The q1..q4 + concat-conv chain is algebraically collapsed into ONE
composed 9x9 conv on qkv0 (conv3x3(W_g, conv_K(V_g, x)) == conv_{K+2}(W_g *
V_g, x); all path kernels share the same 9x9 support, so their sum is a
single kernel), plus the 3x3 pool group on p: 81+9=90 taps vs the naive
9+25+49+45=128 -> 1.42x fewer PE FLOPs.  Composition is exact on the
interior only (the two-stage form zero-clips the intermediate at the ring
just outside the 64x64 grid); the 1-pixel output border is fixed exactly by
4 precomposed delta-kernel strip convs (27 taps each) plus corner add-backs
(the corner ring positions are double-counted by the row+col strips).
The channel conv never materializes: pooled = wch . s where s[i,tap] =
sum_n a_n * qkv[i, n+tap] via PE transposes + shifted-e matmuls; content
logits are a f16 matmul with exp/accum on Scalar.
"""
import os
import sys

for _p in ("/opt/trn_rl_repo", os.path.expanduser("~/.axon_site/_ro/trn_rl_repo")):
    if os.path.isdir(_p) and _p not in sys.path:
        sys.path.append(_p)

import numpy as np
import ml_dtypes
from contextlib import ExitStack

import concourse.bass as bass
import concourse.tile as tile
from concourse import mybir
from concourse import bass_utils

BF16 = mybir.dt.bfloat16
F16 = mybir.dt.float16
F32 = mybir.dt.float32
AF = mybir.ActivationFunctionType
ALU = mybir.AluOpType

B, C, H, W = 16, 256, 128, 128
H2, W2 = 64, 64
N_CORES = 8
BPC = B // N_CORES  # images per core
EPS = 1e-5

# ---------------------------------------------------------------------------
# walrus CoreV3 rejects instructions with more than a couple of sync waits;
# Tile's exit drain accumulates one wait per processor used.  Split the waits
# across a chain of drain instructions (sync engine executes them in order).
# ---------------------------------------------------------------------------
import bass_rust as _br
import concourse.tile as _tile_mod

def _split_drain_and_barrier(self, tick_clock, wait_clock):
    nc = self.nc
    drain_inst = nc.sync.drain()
    wait_clock.add_sem_waits(
        drain_inst.ins, _tile_mod.ScopedClock({None: tick_clock.global_clock})
    )
    W_ = list(drain_inst.ins.sync_info.on_wait)
    if len(W_) > 1:
        drain_inst.ins.sync_info.on_wait = W_[:1]
        for i in range(1, len(W_)):
            extra = nc.sync.drain()
            extra.ins.sync_info = _br.SyncInfo(on_wait=W_[i : i + 1], on_update=[])
    nc.all_engine_barrier()
    assert self.sems is not None
    popped = nc._tile_sem_poison_stack.pop()
    assert popped is self._sem_poison
    nc.clear_and_free_semaphores(list(self.sems.allocated().values()))
    nc.all_engine_barrier()

tile.TileContext._drain_and_barrier = _split_drain_and_barrier

# Same hardware limit applies to scheduled body instructions (max 2 sync waits
# per instruction).  Before lowering, move excess waits onto injected NOPs on
# the same engine.
_MAX_W = 1
_orig_lower_ordered = tile.TileContext._lower_ordered_insts

def _lower_with_wait_split(self, ordered):
    for _bb, insts in ordered.items():
        out = []
        for inst in insts:
            si = getattr(inst, "sync_info", None)
            if si is not None and len(si.on_wait) > _MAX_W:
                wl = list(si.on_wait)
                extra, keep = wl[:-_MAX_W], wl[-_MAX_W:]
                si.on_wait = keep
                for i in range(0, len(extra), _MAX_W):
                    nop = mybir.InstNoOp(
                        name=f"{inst.name}-wsplit{i}",
                        sync_info=mybir.SyncInfo(
                            on_wait=extra[i : i + _MAX_W], on_update=[]
                        ),
                        bass_nofuse=True,
                        engine=inst.engine,
                    )
                    out.append(nop)
            out.append(inst)
        insts[:] = out
    return _orig_lower_ordered(self, ordered)

tile.TileContext._lower_ordered_insts = _lower_with_wait_split


# ---------------------------------------------------------------------------
# correction metadata: one entry per correction lhsT tile, shared by the host
# packer and the kernel emitter.  Bank slot = strip*2 + mt; strips: 0=top,
# 1=bottom, 2=left(+TL/BL corners), 3=right(+TR/BR corners).
# ---------------------------------------------------------------------------
def _corr_meta():
    m = []
    for dy in range(3):
        for kc in range(9):
            m.append(dict(s=0, kind="row", row=4 + dy, c0=kc, a="Dt", i=dy, j=kc))
    for dy in range(3):
        for kc in range(9):
            m.append(dict(s=1, kind="row", row=67 - dy, c0=kc, a="Db", i=dy, j=kc))
    for s, dk, ck1, ck2 in ((2, "Dl", "tl", "bl"), (3, "Dr", "tr", "br")):
        col = (lambda dxc: 4 + dxc) if s == 2 else (lambda dxc: 67 - dxc)
        strips = [dict(s=s, kind="col", r0=dyi, col=col(dxc), a=dk, i=dyi, j=dxc)
                  for dyi in range(9) for dxc in range(3)]
        m.append(strips[0])
        for dy in range(3):
            for dx in range(3):
                m.append(dict(s=s, kind="pt", row=4 + dy, col=col(dx), d0=0,
                              a=ck1, i=dy, j=dx))
                m.append(dict(s=s, kind="pt", row=67 - dy, col=col(dx), d0=63,
                              a=ck2, i=dy, j=dx))
        m.extend(strips[1:])
    first, last = {}, {}
    for idx, e in enumerate(m):
        first.setdefault(e["s"], idx)
        last[e["s"]] = idx
    return m, first, last

CORR_META, CORR_FIRST, CORR_LAST = _corr_meta()


# ---------------------------------------------------------------------------
# host-side weight packing
# ---------------------------------------------------------------------------
def _pack_conv(w, scale=1.0):
    """[O, I, K, K] -> [n_kt, K*K, kt_size, O]  (lhsT blocks per ktile/tap)."""
    O, I, K, _ = w.shape
    kt = 128 if I >= 128 else I
    nkt = I // kt
    a = (np.asarray(w, np.float32) * scale).transpose(1, 2, 3, 0)  # [I,K,K,O]
    a = a.reshape(nkt, kt, K, K, O).transpose(0, 2, 3, 1, 4)
    return np.ascontiguousarray(a.reshape(nkt, K * K, kt, O))


def _f16(a):
    return np.asarray(a).astype(np.float16)


def _pack_conv5(w, scale=1.0):
    """[O, I, K, K] -> [n_kt, K*K, n_mt, kt, 128]: per (ktile, tap, mtile)
    contiguous lhsT blocks for the streaming conv passes."""
    a = _pack_conv(w, scale)  # [nkt, KK, kt, O]
    nkt, kk, kt, O = a.shape
    return np.ascontiguousarray(
        a.reshape(nkt, kk, kt, O // 128, 128).transpose(0, 1, 3, 2, 4)
    )


def _build_composed(inp):
    """Composed 9x9 kernel on qkv0: all concat-conv paths share its support."""
    wcat = np.asarray(inp["conv1x1_w"], np.float32)
    conv1 = np.asarray(inp["conv1_w"], np.float32)[:, :, 0, 0]
    K9 = np.zeros((256, 256, 9, 9), np.float32)
    g0 = wcat[:, 0:256].copy()
    for u in range(3):
        for v in range(3):
            g0[:, :, u, v] += wcat[:, 256:512, u, v] @ conv1
    K9[:, :, 3:6, 3:6] += g0
    for g, wname in ((2, "conv2_w"), (3, "conv3_w"), (4, "conv4_w")):
        V = np.asarray(inp[wname], np.float32)
        Wg = wcat[:, g * 256:(g + 1) * 256]
        Kg = V.shape[2]
        off = (9 - (Kg + 2)) // 2
        Vf = V.reshape(256, -1)
        for u in range(3):
            for v in range(3):
                K9[:, :, off + u:off + u + Kg, off + v:off + v + Kg] += (
                    Wg[:, :, u, v] @ Vf).reshape(256, 256, Kg, Kg)
    return K9


def _build_deltas(inp):
    """Border strip delta kernels (exact fix for the composed conv's border:
    the two-stage form zero-clips the intermediate at ring -1/64)."""
    wcat = np.asarray(inp["conv1x1_w"], np.float32)
    Dt = np.zeros((256, 256, 3, 9), np.float32)
    Db = np.zeros((256, 256, 3, 9), np.float32)
    Dl = np.zeros((256, 256, 9, 3), np.float32)
    Dr = np.zeros((256, 256, 9, 3), np.float32)
    Cc = {k: np.zeros((256, 256, 3, 3), np.float32) for k in ("tl", "tr", "bl", "br")}
    for g, wname in ((2, "conv2_w"), (3, "conv3_w"), (4, "conv4_w")):
        V = np.asarray(inp[wname], np.float32)
        Wg = wcat[:, g * 256:(g + 1) * 256]
        h = (V.shape[2] - 1) // 2
        for u in range(3):
            for dy in range(h):
                for kx in range(2 * h + 1):
                    dx = (u - 1) + (kx - h)
                    Dt[:, :, dy, dx + 4] += Wg[:, :, 0, u] @ V[:, :, dy + h + 1, kx]
                    Db[:, :, dy, dx + 4] += Wg[:, :, 2, u] @ V[:, :, h - 1 - dy, kx]
        for t in range(3):
            for ky in range(2 * h + 1):
                dy = (t - 1) + (ky - h)
                for dxc in range(h):
                    Dl[:, :, dy + 4, dxc] += Wg[:, :, t, 0] @ V[:, :, ky, dxc + h + 1]
                    Dr[:, :, dy + 4, dxc] += Wg[:, :, t, 2] @ V[:, :, ky, h - 1 - dxc]
        for dy in range(h):
            for dx in range(h):
                Cc["tl"][:, :, dy, dx] += Wg[:, :, 0, 0] @ V[:, :, dy + h + 1, dx + h + 1]
                Cc["tr"][:, :, dy, dx] += Wg[:, :, 0, 2] @ V[:, :, dy + h + 1, h - 1 - dx]
                Cc["bl"][:, :, dy, dx] += Wg[:, :, 2, 0] @ V[:, :, h - 1 - dy, dx + h + 1]
                Cc["br"][:, :, dy, dx] += Wg[:, :, 2, 2] @ V[:, :, h - 1 - dy, h - 1 - dx]
    # corner kernels add back the double-subtracted corner ring terms -> negate
    return dict(Dt=Dt, Db=Db, Dl=Dl, Dr=Dr,
                tl=-Cc["tl"], tr=-Cc["tr"], bl=-Cc["bl"], br=-Cc["br"])


def _prep_inputs(inp):
    """Full problem inputs -> dict of packed host arrays (shared by cores)."""
    d = {}
    # reduce conv: fold BN, duplicate output channels to fill 128 partitions
    sc = np.asarray(inp["bn_g"], np.float32) / np.sqrt(
        np.asarray(inp["bn_var"], np.float32) + EPS
    )
    w_red = np.asarray(inp["reduce_w"], np.float32)[:, :, 0, 0] * sc[:, None]  # [64,256]
    b_red = (
        np.asarray(inp["reduce_b"], np.float32) - np.asarray(inp["bn_mean"], np.float32)
    ) * sc + np.asarray(inp["bn_b"], np.float32)
    w_red2 = np.concatenate([w_red, w_red], axis=0)  # [128, 256]
    d["wred"] = _f16(_pack_conv(w_red2[:, :, None, None]))  # [2,1,128,128]
    d["bred"] = np.concatenate([b_red, b_red])[:, None].astype(np.float32)  # [128,1]

    # composed 9x9 (input is M = 2*qkv0 -> fold 0.5) + pool 3x3 group
    K9 = _build_composed(inp)
    wcat = np.asarray(inp["conv1x1_w"], np.float32)
    comb = _pack_conv5(K9, 0.5)                      # [2,81,2,128,128]
    pool = _pack_conv5(wcat[:, 5 * 256:6 * 256])     # [2, 9,2,128,128]
    d["wcomb"] = _f16(np.concatenate([comb, pool], axis=1))  # [2,90,2,128,128]

    # border correction lhsT stream (0.5 folded: reads M off the q0 grid)
    arrs = _build_deltas(inp)
    tiles = []
    for e in CORR_META:
        D = arrs[e["a"]][:, :, e["i"], e["j"]] * 0.5  # [o, i]
        t = D.transpose(1, 0).reshape(2, 128, 2, 128).transpose(0, 2, 1, 3)
        tiles.append(t)
    d["wcorr"] = _f16(np.stack(tiles))  # [144, 2, 2, 128, 128]

    d["wch"] = _f16(_pack_conv5(inp["channel_conv_w"]))  # [2,9,2,128,128]
    wc = np.asarray(inp["conv_w"], np.float32)[0, :, 0, 0]  # [256]
    d["wcont16"] = _f16(wc.reshape(2, 128, 1))  # [2,128,1]
    d["wproj"] = _f16(_pack_conv(inp["proj_w"]))  # [2,1,128,256]
    d["wct1"] = _pack_conv(inp["ct1_w"]).astype(np.float32)  # [2,1,128,32]
    d["ct1b"] = np.asarray(inp["ct1_b"], np.float32)[:, None]  # [32,1]
    d["wct2"] = _pack_conv(inp["ct2_w"]).astype(np.float32)  # [1,1,32,256]
    d["ct2b"] = np.asarray(inp["ct2_b"], np.float32).reshape(2, 128, 1)
    d["ident"] = np.eye(128, dtype=np.float16)
    d["lng"] = np.asarray(inp["ln_g"], np.float32)[:, None]
    d["lnb"] = np.asarray(inp["ln_b"], np.float32)[:, None]
    return d


# ---------------------------------------------------------------------------
# kernel body
# ---------------------------------------------------------------------------
def _emit(nc, tc, ap):
    ctx = ExitStack()
    consts = ctx.enter_context(tc.tile_pool(name="consts", bufs=1))
    acts = ctx.enter_context(tc.tile_pool(name="acts", bufs=1))
    wring = ctx.enter_context(tc.tile_pool(name="wring", bufs=1))
    work = ctx.enter_context(tc.tile_pool(name="work", bufs=1))
    psum = ctx.enter_context(tc.tile_pool(name="psum", bufs=8, space="PSUM"))

    def cst(name, shape, dtype, src):
        t = consts.tile(shape, dtype, tag=name, name=name)
        nc.sync.dma_start(out=t, in_=src)
        return t

    wred = [cst(f"wred{k}", [128, 128], F16, ap["wred"][k, 0]) for k in range(2)]
    bred = cst("bred", [128, 1], F32, ap["bred"])
    wcont16 = [cst(f"wcont16{k}", [128, 1], F16, ap["wcont16"][k]) for k in range(2)]
    wproj = [cst(f"wproj{k}", [128, 256], F16, ap["wproj"][k, 0]) for k in range(2)]
    wct1 = [cst(f"wct1{k}", [128, 32], F32, ap["wct1"][k, 0]) for k in range(2)]
    wct2 = cst("wct2", [32, 256], F32, ap["wct2"][0, 0])
    ct1b = cst("ct1b", [32, 1], F32, ap["ct1b"])
    ct2b = [cst(f"ct2b{k}", [128, 1], F32, ap["ct2b"][k]) for k in range(2)]
    ident = cst("ident", [128, 128], F16, ap["ident"])
    lng = cst("lng", [32, 1], F32, ap["lng"])
    lnb = cst("lnb", [32, 1], F32, ap["lnb"])

    sigma = consts.tile([128, 1], F32, tag="sigma", name="sigma")
    nc.vector.memset(sigma[0:64, :], 1.0)
    nc.vector.memset(sigma[64:128, :], -1.0)
    epsv = consts.tile([32, 1], F32, tag="epsv", name="epsv")
    nc.vector.memset(epsv, EPS)
    onesb = consts.tile([1, 128], BF16, tag="onesb", name="onesb")
    nc.vector.memset(onesb, 1.0)
    onesf = consts.tile([32, 1], F32, tag="onesf", name="onesf")
    nc.vector.memset(onesf, 1.0)
    onesf2 = consts.tile([1, 32], F32, tag="onesf2", name="onesf2")
    nc.vector.memset(onesf2, 1.0)

    # ---- activation buffers (f16).  q0/pb are per-image (phase-1 of image
    # i+1 overlaps image i's conv pass); qkv is shared (WAR deps order the
    # next image's drains behind the previous image's tail reads).
    def padbuf(name, hw):
        return acts.tile([128, hw, hw], F16, tag=name, name=name)

    # q0 holds both images in one tile per input-channel half so the border
    # correction can matmul both images' strips with a single lhsT load.
    q0k = [
        acts.tile([128, BPC, 72, 72], F16, tag=f"q0_{k}", name=f"q0_{k}")
        for k in range(2)
    ]
    q0 = [[q0k[k][:, im] for k in range(2)] for im in range(BPC)]
    pb = [[padbuf(f"p_{im}_{k}", 66) for k in range(2)] for im in range(BPC)]
    qkv_t = [
        [
            acts.tile([128, 4480], F16, tag=f"qkv_{im}_{k}", name=f"qkv_{im}_{k}")
            for k in range(2)
        ]
        for im in range(BPC)
    ]
    qkv = [
        [t[:, 0:4356].rearrange("p (a b) -> p a b", b=66) for t in qkv_t[im]]
        for im in range(BPC)
    ]
    for im in range(BPC):
        for t in qkv_t[im]:
            nc.gpsimd.memset(t[:, 4356:4480], 0.0)

    # zero the halo borders once (interior is fully overwritten per image)
    for t in q0k:
        nc.gpsimd.memset(t[:, :, 0:4, :], 0.0)
        nc.gpsimd.memset(t[:, :, 68:72, :], 0.0)
        nc.gpsimd.memset(t[:, :, 4:68, 0:4], 0.0)
        nc.gpsimd.memset(t[:, :, 4:68, 68:72], 0.0)
    for t in [t_ for im in range(BPC) for t_ in pb[im]] + [
        t_ for im in range(BPC) for t_ in qkv[im]
    ]:
        nc.gpsimd.memset(t[:, 0:1, :], 0.0)
        nc.gpsimd.memset(t[:, 65:66, :], 0.0)
        nc.gpsimd.memset(t[:, 1:65, 0:1], 0.0)
        nc.gpsimd.memset(t[:, 1:65, 65:66], 0.0)

    # ---- phase 1: stream x -> reduce conv+ReLU -> DWT -> M;  maxpool -> p
    class Ph1:
        def __init__(self, img):
            self.img = img
            self.xts = []
            self.sc = 0

        def emit_dmas(self, scs=range(16)):
            for sc_ in scs:
                pair = []
                for k in range(2):
                    xt = work.tile(
                        [128, 8, 128], F16, tag=f"x{k}", bufs=8, name=f"xt{k}"
                    )
                    src = ap["x"][
                        self.img, k * 128 : (k + 1) * 128, sc_ * 8 : sc_ * 8 + 8, :
                    ]
                    nc.sync.dma_start(out=xt, in_=src)
                    pair.append(xt)
                self.xts.append(pair)

        def step(self, n=1):
            for _ in range(n):
                if self.sc < 16:
                    self._sc(self.sc)
                    self.sc += 1

        def _sc(self, sc_):
            xts = self.xts[sc_]
            orow = sc_ * 4  # 8 input rows -> 4 output rows
            rch = work.tile([128, 8, 128], F16, tag="rch", bufs=2, name="rch")
            for sub in range(2):
                ps = psum.tile([128, 4, 128], F32, tag="ps", name="ps_r")
                for k in range(2):
                    nc.tensor.matmul(
                        ps, wred[k], xts[k][:, sub * 4 : sub * 4 + 4, :],
                        start=(k == 0), stop=(k == 1),
                    )
                nc.scalar.activation(
                    out=rch[:, sub * 4 : sub * 4 + 4, :], in_=ps, func=AF.Relu,
                    bias=bred, scale=1.0,
                )
            rv = rch.rearrange("p (a two) (c cp) -> p a two c cp", two=2, cp=2)
            a_, b_ = rv[:, :, 0, :, 0], rv[:, :, 0, :, 1]
            c_, d_ = rv[:, :, 1, :, 0], rv[:, :, 1, :, 1]
            u = work.tile([128, 4, 64], F32, tag="u", bufs=2, name="u")
            v = work.tile([128, 4, 64], F32, tag="v", bufs=2, name="v")
            s_ = work.tile([128, 4, 64], F32, tag="s", bufs=2, name="s_")
            t_ = work.tile([128, 4, 64], F32, tag="t", bufs=2, name="t_")
            nc.vector.tensor_add(u, a_, b_)
            nc.vector.tensor_add(v, c_, d_)
            nc.vector.tensor_sub(s_, a_, b_)
            nc.vector.tensor_sub(t_, c_, d_)
            sv = work.tile([128, 4, 64], F32, tag="sv", bufs=2, name="sv")
            st = work.tile([128, 4, 64], F32, tag="st", bufs=2, name="st")
            # sigma-scale on the Scalar engine to unload DVE
            nc.scalar.activation(out=sv, in_=v, func=AF.Copy, scale=sigma)
            nc.scalar.activation(out=st, in_=t_, func=AF.Copy, scale=sigma)
            myq0 = q0[self.img]
            nc.vector.tensor_add(myq0[0][:, 4 + orow : 8 + orow, 4:68], u, sv)
            nc.vector.tensor_add(myq0[1][:, 4 + orow : 8 + orow, 4:68], s_, st)
            for k in range(2):
                xv = xts[k].rearrange("p (a two) (c cp) -> p a two c cp", two=2, cp=2)
                xa, xb = xv[:, :, 0, :, 0], xv[:, :, 0, :, 1]
                xc, xd = xv[:, :, 1, :, 0], xv[:, :, 1, :, 1]
                m1 = work.tile([128, 4, 64], F16, tag="m1", bufs=1, name="m1")
                m2 = work.tile([128, 4, 64], F16, tag="m2", bufs=1, name="m2")
                nc.vector.tensor_max(m1, xa, xb)
                nc.vector.tensor_max(m2, xc, xd)
                nc.vector.tensor_max(pb[self.img][k][:, 1 + orow : 5 + orow, 1:65], m1, m2)

    # ---- main pass: accumulate all 90 taps x 2 ktiles into 8 psum banks
    # (2 mt x 4 chunks of 8 rows) per spatial half.
    def conv_pass(img, boundary_hooks, inloop=None):
        g0t = q0[img]
        pbt = pb[img]
        for qh in range(2):
            pss = [
                [psum.tile([128, 8, 64], F32, tag="ps", name="ps_c") for _ in range(4)]
                for _ in range(2)
            ]
            n = 0
            for ik in range(2):
                for bq in range(15):
                    if inloop and qh == 0:
                        fn = inloop.pop(ik * 15 + bq, None)
                        if fn:
                            fn()
                    wt = wring.tile([128, 1536], F16, tag="wtap6", bufs=3, name="wt")
                    nc.sync.dma_start(
                        out=wt,
                        in_=ap["wcomb"][ik, bq * 6 : bq * 6 + 6].rearrange(
                            "t m p c -> p (t m) c"
                        ),
                    )
                    for ti in range(6):
                        tp = bq * 6 + ti
                        for mt in range(2):
                            lhsT = wt[:, (ti * 2 + mt) * 128 : (ti * 2 + mt + 1) * 128]
                            for ci in range(4):
                                r0 = qh * 32 + ci * 8
                                if tp < 81:
                                    ky, kx = tp // 9, tp % 9
                                    rhs = g0t[ik][:, ky + r0 : ky + r0 + 8, kx : kx + 64]
                                else:
                                    tpp = tp - 81
                                    ky, kx = tpp // 3, tpp % 3
                                    rhs = pbt[ik][:, ky + r0 : ky + r0 + 8, kx : kx + 64]
                                nc.tensor.matmul(
                                    pss[mt][ci], lhsT, rhs,
                                    start=(n == 0), stop=(n == 179),
                                )
                        n += 1
            for ci in range(4):
                r0 = qh * 32 + ci * 8
                nc.vector.tensor_copy(qkv[img][0][:, 1 + r0 : 9 + r0, 1:65], pss[0][ci])
                nc.scalar.copy(qkv[img][1][:, 1 + r0 : 9 + r0, 1:65], pss[1][ci])
            boundary_hooks[qh]()

    # ---- exact border correction: 4 strip convs + corner add-backs into one
    # psum bank per image (both images share each streamed weight tile -> the
    # 18MB correction stream is paid once), then subtract from qkv borders.
    corr_sbs = [None, None]

    def corr_accum_both():
        # bank1: top/bottom strips, bank2: left/right (+corner pts); each
        # matmul covers BOTH images via the combined q0 layout (free = 2x64).
        bank1 = psum.tile([128, BPC, 4, 64], F32, tag="ps", name="corr_ps1")
        bank2 = psum.tile([128, BPC, 4, 64], F32, tag="ps", name="corr_ps2")
        for bi in range(36):
          wt = wring.tile([128, 2048], F16, tag="wcorrw4", bufs=2, name="wc")
          nc.sync.dma_start(
              out=wt,
              in_=ap["wcorr"][bi * 4 : bi * 4 + 4].rearrange(
                  "t k m p c -> p (t k m) c"
              ),
          )
          for ti in range(4):
            idx = bi * 4 + ti
            e = CORR_META[idx]
            s = e["s"]
            if idx == CORR_FIRST[s]:
                # chain opener: start=True matmul over a guaranteed-zero region
                # of the padded grid (rows/cols 0..3 are zero), so every real
                # tap accumulates (start=True contributions land unreliably on
                # sub-bank regions).
                for mt in range(2):
                    if e["kind"] == "row":
                        z_rhs = q0k[0][:, :, 0:1, 0:64]
                        z_dst = bank1[:, :, s * 2 + mt : s * 2 + mt + 1, :]
                    else:
                        z_rhs = q0k[0][:, :, 0:64, 0:1]
                        sl = (s - 2) * 2 + mt
                        z_dst = bank2[:, :, sl : sl + 1, :].rearrange(
                            "p a b c -> p a c b"
                        )
                    nc.tensor.matmul(
                        z_dst, wt[:, (ti * 4 + mt) * 128 : (ti * 4 + mt + 1) * 128],
                        z_rhs, start=True, stop=False,
                    )
            for ik in range(2):
                if e["kind"] == "row":
                    rhs = q0k[ik][:, :, e["row"] : e["row"] + 1, e["c0"] : e["c0"] + 64]
                    dsts = [bank1[:, :, s * 2 + mt : s * 2 + mt + 1, :]
                            for mt in range(2)]
                elif e["kind"] == "col":
                    rhs = q0k[ik][:, :, e["r0"] : e["r0"] + 64, e["col"] : e["col"] + 1]
                    dsts = [bank2[:, :, (s - 2) * 2 + mt : (s - 2) * 2 + mt + 1, :]
                            .rearrange("p a b c -> p a c b") for mt in range(2)]
                else:
                    rhs = q0k[ik][:, :, e["row"] : e["row"] + 1, e["col"] : e["col"] + 1]
                    dsts = [bank2[:, :, (s - 2) * 2 + mt : (s - 2) * 2 + mt + 1,
                                  e["d0"] : e["d0"] + 1] for mt in range(2)]
                for mt in range(2):
                    j = ti * 4 + ik * 2 + mt
                    nc.tensor.matmul(
                        dsts[mt],
                        wt[:, j * 128 : (j + 1) * 128],
                        rhs,
                        start=False,
                        stop=(idx == CORR_LAST[s] and ik == 1),
                    )
        for bn, bank in ((0, bank1), (1, bank2)):
            sb = work.tile([128, BPC, 4, 64], F32, tag=f"corr{bn}", name=f"corr_sb{bn}")
            nc.scalar.copy(sb, bank)
            corr_sbs[bn] = sb

    def corr_apply(img):
        sb1 = corr_sbs[0][:, img]
        sb2 = corr_sbs[1][:, img]
        for mt in range(2):
            qv = qkv[img][mt]
            nc.vector.tensor_sub(
                qv[:, 1:2, 1:65], qv[:, 1:2, 1:65], sb1[:, 0 + mt : 1 + mt, :]
            )
            nc.vector.tensor_sub(
                qv[:, 64:65, 1:65], qv[:, 64:65, 1:65], sb1[:, 2 + mt : 3 + mt, :]
            )
            nc.vector.tensor_sub(
                qv[:, 1:65, 1:2], qv[:, 1:65, 1:2],
                sb2[:, 0 + mt : 1 + mt, :].rearrange("p a b -> p b a"),
            )
            nc.vector.tensor_sub(
                qv[:, 1:65, 64:65], qv[:, 1:65, 64:65],
                sb2[:, 2 + mt : 3 + mt, :].rearrange("p a b -> p b a"),
            )

    # ---- content logits + softmax numerator/denominator on the (corrected)
    # qkv tiles; e-maps bounced to DRAM scratch for the shifted gathers.
    def content_pass(img):
        for ci in range(8):
            r0 = ci * 8
            cp = psum.tile([1, 8, 64], F32, tag="ps", name="cp")
            for k in range(2):
                nc.tensor.matmul(
                    cp, wcont16[k], qkv[img][k][:, 1 + r0 : 9 + r0, 1:65],
                    start=(k == 0), stop=(k == 1),
                )
            nc.scalar.activation(
                out=e66i[:, 1 + r0 : 9 + r0, 1:65], in_=cp, func=AF.Exp,
                bias=0.0, scale=1.0, accum_out=denc[:, ci : ci + 1],
            )
        # bounce the whole e-grid once, then 9 full-width shifted gathers
        lo = 134 + 66
        nc.sync.dma_start(
            out=ap["e_scr"][:, lo : lo + 64 * 66], in_=e66_sb[:, lo : lo + 64 * 66]
        )
        for k9 in range(9):
            ky, kx = k9 // 3, k9 % 3
            off = 201 - 66 * ky - kx
            nc.sync.dma_start(
                out=aT[:, k9, :],
                in_=ap["e_scr"][:, off : off + 4480].rearrange(
                    "o (p t) -> (o p) t", t=35
                ),
            )

    # ---- attention tail part A: softmax denominator, then the attention
    # pooling s[i,tap] = sum_n e_n * qkv[i, n+tap] entirely on the PE:
    # transpose qkv spatial->partition tile by tile, contract against 9
    # shifted (unnormalized) e-maps gathered by strided DMA, normalize by
    # 1/den when casting s to f16, transpose s back to channel-major.
    def tail_a(img):
        dent = work.tile([1, 1], F32, tag="dent", name="dent")
        nc.vector.tensor_reduce(
            dent, denc, axis=mybir.AxisListType.X, op=ALU.add
        )
        rden = work.tile([1, 1], F32, tag="rden", name="rden")
        nc.vector.reciprocal(rden, dent)
        rden_bf = work.tile([1, 1], BF16, tag="rden_bf", name="rden_bf")
        nc.vector.tensor_copy(rden_bf, rden)
        rd_ps = psum.tile([128, 1], F32, tag="ps", name="rd_ps")
        nc.tensor.matmul(rd_ps, onesb, rden_bf, start=True, stop=True)
        rdsb = work.tile([128, 1], F32, tag="rdsb", name="rdsb")
        nc.vector.tensor_copy(rdsb, rd_ps)
        # 9 shifted views of the padded e-grid, spatial-on-partitions with
        # m = p*35 + t so every partition reads one contiguous 70B run.
        # (SBUF partition dims can't alias free-dim strides, so bounce the
        # flat e-grid through DRAM scratch and gather from there.)
        qv35 = [
            qkv_t[img][ik].rearrange("p (m t) -> p t m", t=35) for ik in range(2)
        ]
        smm = psum.tile([9, 256], F32, tag="ps", name="smm")
        for t in range(35):
            qpT = work.tile([128, 2, 128], BF16, tag="qpT", bufs=4, name="qpT")
            for ik in range(2):
                tp_ps = psum.tile([128, 128], F16, tag="ps", name="tp_ps")
                nc.tensor.transpose(tp_ps, qv35[ik][:, t, :], ident)
                if ik == 0:
                    nc.scalar.copy(qpT[:, ik, :], tp_ps)
                else:
                    nc.vector.tensor_copy(qpT[:, ik, :], tp_ps)
            nc.tensor.matmul(
                smm, aT[:, :, t], qpT.rearrange("p a b -> p (a b)"),
                start=(t == 0), stop=(t == 34),
            )
        s_sb = work.tile([9, 256], F16, tag="s_sb", name="s_sb")
        nc.scalar.activation(
            out=s_sb, in_=smm, func=AF.Copy, scale=rdsb[0:9, :]
        )
        for ik in range(2):
            st_ps = psum.tile([128, 9], F16, tag="ps", name="st_ps")
            nc.tensor.transpose(
                st_ps, s_sb[:, ik * 128 : (ik + 1) * 128], ident[0:9, 0:9]
            )
            nc.scalar.copy(s16[ik], st_ps)

    # ---- attention tail part B: pooled matvec, channel transform, proj conv
    def tail_b(img):
        pooled_ps = [
            psum.tile([128, 1], F32, tag="ps", name=f"pool_ps{mt}") for mt in range(2)
        ]
        for ik in range(2):
            wt = wring.tile([128, 2304], F16, tag="wchw", bufs=1, name="wtc")
            nc.sync.dma_start(
                out=wt, in_=ap["wch"][ik].rearrange("t m p c -> p (t m) c")
            )
            for tp in range(9):
                for mt in range(2):
                    nc.tensor.matmul(
                        pooled_ps[mt],
                        wt[:, (tp * 2 + mt) * 128 : (tp * 2 + mt + 1) * 128],
                        s16[ik][:, tp : tp + 1],
                        start=(ik == 0 and tp == 0), stop=(ik == 1 and tp == 8),
                    )
        pooled = []
        for mt in range(2):
            pl = work.tile([128, 1], F32, tag=f"pool{mt}", name="pl")
            nc.vector.tensor_copy(pl, pooled_ps[mt])
            pooled.append(pl)

        # channel transform (tiny, fp32)
        t_ps = psum.tile([32, 1], F32, tag="ps", name="t_ps")
        for k in range(2):
            nc.tensor.matmul(t_ps, wct1[k], pooled[k], start=(k == 0), stop=(k == 1))
        ts2 = work.tile([32, 2], F32, tag="ts2", name="ts2")
        t_sb = ts2[:, 0:1]
        nc.vector.tensor_scalar_add(t_sb, t_ps, ct1b)
        nc.vector.tensor_mul(ts2[:, 1:2], t_sb, t_sb)
        sums_ps = psum.tile([1, 2], F32, tag="ps", name="sums_ps")
        nc.tensor.matmul(sums_ps, onesf, ts2, start=True, stop=True)
        sums_sb = work.tile([1, 2], F32, tag="sums_sb", name="sums_sb")
        nc.vector.tensor_copy(sums_sb, sums_ps)
        bc_ps = psum.tile([32, 2], F32, tag="ps", name="bc_ps")
        nc.tensor.matmul(bc_ps, onesf2, sums_sb, start=True, stop=True)
        mean = work.tile([32, 1], F32, tag="mean", name="mean")
        nc.vector.tensor_scalar_mul(mean, bc_ps[:, 0:1], 1.0 / 32)
        mv = work.tile([32, 1], F32, tag="mv", name="mv")
        nc.vector.tensor_scalar_mul(mv, bc_ps[:, 1:2], 1.0 / 32)
        m2t = work.tile([32, 1], F32, tag="m2t", name="m2t")
        nc.vector.tensor_mul(m2t, mean, mean)
        var = work.tile([32, 1], F32, tag="var", name="var")
        nc.vector.tensor_sub(var, mv, m2t)
        sd = work.tile([32, 1], F32, tag="sd", name="sd")
        nc.scalar.activation(out=sd, in_=var, func=AF.Sqrt, bias=epsv, scale=1.0)
        rsd = work.tile([32, 1], F32, tag="rsd", name="rsd")
        nc.vector.reciprocal(rsd, sd)
        dt_ = work.tile([32, 1], F32, tag="dt", name="dt_")
        nc.vector.tensor_sub(dt_, t_sb, mean)
        tn = work.tile([32, 1], F32, tag="tn", name="tn")
        nc.vector.tensor_mul(tn, dt_, rsd)
        tact = work.tile([32, 1], F32, tag="tact", name="tact")
        nc.scalar.activation(out=tact, in_=tn, func=AF.Relu, bias=lnb, scale=lng)

        projs = []
        for mt in range(2):
            cw_ps = psum.tile([128, 1], F32, tag="ps", name="cw_ps")
            nc.tensor.matmul(
                cw_ps, wct2[:, mt * 128 : (mt + 1) * 128], tact, start=True, stop=True
            )
            cw = work.tile([128, 1], F32, tag=f"cw{mt}", name="cw")
            nc.vector.tensor_scalar_add(cw, cw_ps, ct2b[mt])
            pj = work.tile([128, 256], F16, tag=f"projs{mt}", name="pj")
            nc.vector.tensor_scalar_mul(pj, wproj[mt], cw)
            projs.append(pj)

        # out = proj(qkv * cw)  (cw folded into proj weights)
        for mt in range(2):
            for ci in range(8):
                r0 = ci * 8
                po = psum.tile([128, 8, 64], F32, tag="ps", name="po")
                for k in range(2):
                    nc.tensor.matmul(
                        po,
                        projs[k][:, mt * 128 : (mt + 1) * 128],
                        qkv[img][k][:, 1 + r0 : 9 + r0, 1:65],
                        start=(k == 0), stop=(k == 1),
                    )
                ost = work.tile([128, 8, 64], F16, tag="ost", bufs=2, name="ost")
                if ci % 2 == 0:
                    nc.scalar.copy(ost, po)
                else:
                    nc.vector.tensor_copy(ost, po)
                nc.sync.dma_start(
                    out=ap["out"][img, mt * 128 : (mt + 1) * 128, r0 : r0 + 8, :],
                    in_=ost,
                )

    # shared tail tiles.  e66_sb: e embedded at offset 134 on a 66-wide,
    # zero-bordered grid (margins make all 9 shifted DMA gathers in-range).
    e66_sb = work.tile([1, 4736], BF16, tag="e66", name="e66")
    nc.gpsimd.memset(e66_sb, 0.0)
    e66i = e66_sb[:, 134 : 134 + 4488].rearrange("o (r c) -> o r c", c=66)
    denc = work.tile([1, 8], F32, tag="denc", name="denc")
    aT = work.tile([128, 9, 35], BF16, tag="aT", name="aT")
    nc.sync.dma_start(out=ap["e_scr"], in_=e66_sb)  # zero borders once
    s16 = [work.tile([128, 9], F16, tag=f"s16_{ik}", name="s16") for ik in range(2)]

    # ---- schedule ----
    ph1 = [Ph1(im) for im in range(BPC)]
    ph1[0].emit_dmas()
    ph1[0].step(16)

    def hook0():
        ph1[1].step(8)
        ph1[1].emit_dmas(range(8, 16))

    def hook1():
        ph1[1].step(8)

    def hookA():
        # pass(1) qh0/qh1 boundary: both images' border corrections (shared
        # weight stream), then image 0's whole attention prologue -- the
        # e-map bounce/gather latency and corr-stream gaps cover each other,
        # and qh1 runs while image 0's tail state settles.
        corr_accum_both()
        corr_apply(0)
        if "dbgq" in ap:
            for ik in range(2):
                nc.sync.dma_start(out=ap["dbgq"][0, ik], in_=qkv_t[0][ik][:, 0:4356])
        content_pass(0)
        tail_a(0)

    conv_pass(0, [hook0, hook1],
              inloop={10: lambda: ph1[1].emit_dmas(range(0, 8))})
    conv_pass(1, [hookA, lambda: None])
    corr_apply(1)
    if "dbgq" in ap:
        for ik in range(2):
            nc.sync.dma_start(out=ap["dbgq"][1, ik], in_=qkv_t[1][ik][:, 0:4356])
    content_pass(1)
    tail_b(0)
    tail_a(1)
    tail_b(1)
    ctx.close()
    return None


def build():
    nc = bass.Bass("TRN2", target_bir_lowering=False, debug=False)
    shapes = {
        "x": ([BPC, C, H, W], F16),
        "wred": ([2, 1, 128, 128], F16),
        "bred": ([128, 1], F32),
        "wcomb": ([2, 90, 2, 128, 128], F16),
        "wcorr": ([144, 2, 2, 128, 128], F16),
        "wch": ([2, 9, 2, 128, 128], F16),
        "wcont16": ([2, 128, 1], F16),
        "wproj": ([2, 1, 128, 256], F16),
        "wct1": ([2, 1, 128, 32], F32),
        "ct1b": ([32, 1], F32),
        "wct2": ([1, 1, 32, 256], F32),
        "ct2b": ([2, 128, 1], F32),
        "ident": ([128, 128], F16),
        "lng": ([32, 1], F32),
        "lnb": ([32, 1], F32),
    }
    ap = {
        k: nc.dram_tensor(k, shp, dt, kind="ExternalInput").ap()
        for k, (shp, dt) in shapes.items()
    }
    ap["out"] = nc.dram_tensor("out", [BPC, C, H2, W2], F16, kind="ExternalOutput").ap()
    ap["e_scr"] = nc.dram_tensor("e_scr", [1, 4736], BF16, kind="Internal").ap()
    if os.environ.get("DWT_DBG"):
        ap["dbgq"] = nc.dram_tensor(
            "dbgq", [BPC, 2, 128, 4356], F16, kind="ExternalOutput"
        ).ap()
    with tile.TileContext(nc) as tc:
        _emit(nc, tc, ap)
    return nc


_CACHED_NC = {}


def _install_trace_hook():
    """The image's antenv lacks axon_hooks; shim it and register the boot's
    ctypes NTFF hook so trace=True works.  Also neutralize the S3 artifact
    upload (no bucket access here)."""
    import types
    import antenv

    if "antenv.axon_hooks" not in sys.modules:
        mod = types.ModuleType("antenv.axon_hooks")
        mod._hook = None
        def set_axon_ntff_profile_hook(h):
            mod._hook = h
        def get_axon_ntff_profile_hook():
            return mod._hook
        mod.set_axon_ntff_profile_hook = set_axon_ntff_profile_hook
        mod.get_axon_ntff_profile_hook = get_axon_ntff_profile_hook
        sys.modules["antenv.axon_hooks"] = mod
        antenv.axon_hooks = mod
        from trn_agent_boot.trn_boot import _ntff_profile_via_ctypes
        mod.set_axon_ntff_profile_hook(
            _ntff_profile_via_ctypes("/opt/axon/libaxon_pjrt.so")
        )
        bass_utils.upload_artifacts = lambda tmpdir: tmpdir


def run(inputs, debug=False, trace=False):
    if trace:
        _install_trace_hook()
    if "nc" not in _CACHED_NC:
        _CACHED_NC["nc"] = build()
    nc = _CACHED_NC["nc"]
    d = _prep_inputs(inputs)
    x_f16 = np.asarray(inputs["x"], np.float32).astype(np.float16)
    in_maps = []
    for c in range(N_CORES):
        m = dict(d)
        m["x"] = np.ascontiguousarray(x_f16[c * BPC : (c + 1) * BPC])
        in_maps.append(m)
    res = bass_utils.run_bass_kernel_spmd(
        nc, in_maps, core_ids=list(range(N_CORES)), trace=trace
    )
    out = np.concatenate(
        [res.results[c]["out"].astype(np.float32) for c in range(N_CORES)], axis=0
    )
    return out, res


def kernel(**inputs):
    out, _ = run(inputs)
    return out


# revision 44
# speedup vs baseline: 1.0190x; 1.0190x over previous
"""Trainium2 Bass kernel for nn_Dwtpool (dense_cnn).

Reference graph (per image, C=256, 128x128 input):
  p    = maxpool2x2(x)                          -> [256, 64, 64]
  r    = ReLU(BN(conv1x1(x, reduce_w)))         -> [ 64,128,128]
  M    = haar_dwt(r) * 2  (stored unscaled)     -> [256, 64, 64]
  q2..q4 = conv{3,5,7}(0.5*M)                   -> [256, 64, 64] each
  qkv  = conv3x3(concat[0.5*M, q1..q4, p])      -> [256, 64, 64]
  att  = softmax_spatial(conv1x1(qkv)); pooled = sum_n ch(qkv)_c,n * att_n
  cw   = ct2(ReLU(LN(ct1(pooled))))             -> [256]
  out  = conv1x1(qkv * cw, proj_w)              -> [256, 64, 64]

Strategy: data-parallel over batch (16 images / 8 cores = 2 per core), fp16
trunk.  The q1..q4 + concat-conv chain is algebraically collapsed into ONE
composed 9x9 conv on qkv0 (conv3x3(W_g, conv_K(V_g, x)) == conv_{K+2}(W_g *
V_g, x); all path kernels share the same 9x9 support, so their sum is a
single kernel), plus the 3x3 pool group on p: 81+9=90 taps vs the naive
9+25+49+45=128 -> 1.42x fewer PE FLOPs.  Composition is exact on the
interior only (the two-stage form zero-clips the intermediate at the ring
just outside the 64x64 grid); the 1-pixel output border is fixed exactly by
4 precomposed delta-kernel strip convs (27 taps each) plus corner add-backs
(the corner ring positions are double-counted by the row+col strips).
The channel conv never materializes: pooled = wch . s where s[i,tap] =
sum_n a_n * qkv[i, n+tap] via PE transposes + shifted-e matmuls; content
logits are a f16 matmul with exp/accum on Scalar.
"""
import os
import sys

for _p in ("/opt/trn_rl_repo", os.path.expanduser("~/.axon_site/_ro/trn_rl_repo")):
    if os.path.isdir(_p) and _p not in sys.path:
        sys.path.append(_p)

import numpy as np
import ml_dtypes
from contextlib import ExitStack

import concourse.bass as bass
import concourse.tile as tile
from concourse import mybir
from concourse import bass_utils

BF16 = mybir.dt.bfloat16
F16 = mybir.dt.float16
F32 = mybir.dt.float32
AF = mybir.ActivationFunctionType
ALU = mybir.AluOpType

B, C, H, W = 16, 256, 128, 128
H2, W2 = 64, 64
N_CORES = 8
BPC = B // N_CORES  # images per core
EPS = 1e-5

# ---------------------------------------------------------------------------
# walrus CoreV3 rejects instructions with more than a couple of sync waits;
# Tile's exit drain accumulates one wait per processor used.  Split the waits
# across a chain of drain instructions (sync engine executes them in order).
# ---------------------------------------------------------------------------
import bass_rust as _br
import concourse.tile as _tile_mod

def _split_drain_and_barrier(self, tick_clock, wait_clock):
    nc = self.nc
    drain_inst = nc.sync.drain()
    wait_clock.add_sem_waits(
        drain_inst.ins, _tile_mod.ScopedClock({None: tick_clock.global_clock})
    )
    W_ = list(drain_inst.ins.sync_info.on_wait)
    if len(W_) > 1:
        drain_inst.ins.sync_info.on_wait = W_[:1]
        for i in range(1, len(W_)):
            extra = nc.sync.drain()
            extra.ins.sync_info = _br.SyncInfo(on_wait=W_[i : i + 1], on_update=[])
    nc.all_engine_barrier()
    assert self.sems is not None
    popped = nc._tile_sem_poison_stack.pop()
    assert popped is self._sem_poison
    nc.clear_and_free_semaphores(list(self.sems.allocated().values()))
    nc.all_engine_barrier()

tile.TileContext._drain_and_barrier = _split_drain_and_barrier

# Same hardware limit applies to scheduled body instructions (max 2 sync waits
# per instruction).  Before lowering, move excess waits onto injected NOPs on
# the same engine.
_MAX_W = 1
_orig_lower_ordered = tile.TileContext._lower_ordered_insts

def _lower_with_wait_split(self, ordered):
    for _bb, insts in ordered.items():
        out = []
        for inst in insts:
            si = getattr(inst, "sync_info", None)
            if si is not None and len(si.on_wait) > _MAX_W:
                wl = list(si.on_wait)
                extra, keep = wl[:-_MAX_W], wl[-_MAX_W:]
                si.on_wait = keep
                for i in range(0, len(extra), _MAX_W):
                    nop = mybir.InstNoOp(
                        name=f"{inst.name}-wsplit{i}",
                        sync_info=mybir.SyncInfo(
                            on_wait=extra[i : i + _MAX_W], on_update=[]
                        ),
                        bass_nofuse=True,
                        engine=inst.engine,
                    )
                    out.append(nop)
            out.append(inst)
        insts[:] = out
    return _orig_lower_ordered(self, ordered)

tile.TileContext._lower_ordered_insts = _lower_with_wait_split


# ---------------------------------------------------------------------------
# correction metadata: one entry per correction lhsT tile, shared by the host
# packer and the kernel emitter.  Bank slot = strip*2 + mt; strips: 0=top,
# 1=bottom, 2=left(+TL/BL corners), 3=right(+TR/BR corners).
# ---------------------------------------------------------------------------
def _corr_meta():
    m = []
    for dy in range(3):
        for kc in range(9):
            m.append(dict(s=0, kind="row", row=4 + dy, c0=kc, a="Dt", i=dy, j=kc))
    for dy in range(3):
        for kc in range(9):
            m.append(dict(s=1, kind="row", row=67 - dy, c0=kc, a="Db", i=dy, j=kc))
    for s, dk, ck1, ck2 in ((2, "Dl", "tl", "bl"), (3, "Dr", "tr", "br")):
        col = (lambda dxc: 4 + dxc) if s == 2 else (lambda dxc: 67 - dxc)
        strips = [dict(s=s, kind="col", r0=dyi, col=col(dxc), a=dk, i=dyi, j=dxc)
                  for dyi in range(9) for dxc in range(3)]
        m.append(strips[0])
        for dy in range(3):
            for dx in range(3):
                m.append(dict(s=s, kind="pt", row=4 + dy, col=col(dx), d0=0,
                              a=ck1, i=dy, j=dx))
                m.append(dict(s=s, kind="pt", row=67 - dy, col=col(dx), d0=63,
                              a=ck2, i=dy, j=dx))
        m.extend(strips[1:])
    first, last = {}, {}
    for idx, e in enumerate(m):
        first.setdefault(e["s"], idx)
        last[e["s"]] = idx
    return m, first, last

CORR_META, CORR_FIRST, CORR_LAST = _corr_meta()


# ---------------------------------------------------------------------------
# host-side weight packing
# ---------------------------------------------------------------------------
def _pack_conv(w, scale=1.0):
    """[O, I, K, K] -> [n_kt, K*K, kt_size, O]  (lhsT blocks per ktile/tap)."""
    O, I, K, _ = w.shape
    kt = 128 if I >= 128 else I
    nkt = I // kt
    a = (np.asarray(w, np.float32) * scale).transpose(1, 2, 3, 0)  # [I,K,K,O]
    a = a.reshape(nkt, kt, K, K, O).transpose(0, 2, 3, 1, 4)
    return np.ascontiguousarray(a.reshape(nkt, K * K, kt, O))


def _f16(a):
    return np.asarray(a).astype(np.float16)


def _pack_conv5(w, scale=1.0):
    """[O, I, K, K] -> [n_kt, K*K, n_mt, kt, 128]: per (ktile, tap, mtile)
    contiguous lhsT blocks for the streaming conv passes."""
    a = _pack_conv(w, scale)  # [nkt, KK, kt, O]
    nkt, kk, kt, O = a.shape
    return np.ascontiguousarray(
        a.reshape(nkt, kk, kt, O // 128, 128).transpose(0, 1, 3, 2, 4)
    )


def _build_composed(inp):
    """Composed 9x9 kernel on qkv0: all concat-conv paths share its support."""
    wcat = np.asarray(inp["conv1x1_w"], np.float32)
    conv1 = np.asarray(inp["conv1_w"], np.float32)[:, :, 0, 0]
    K9 = np.zeros((256, 256, 9, 9), np.float32)
    g0 = wcat[:, 0:256].copy()
    for u in range(3):
        for v in range(3):
            g0[:, :, u, v] += wcat[:, 256:512, u, v] @ conv1
    K9[:, :, 3:6, 3:6] += g0
    for g, wname in ((2, "conv2_w"), (3, "conv3_w"), (4, "conv4_w")):
        V = np.asarray(inp[wname], np.float32)
        Wg = wcat[:, g * 256:(g + 1) * 256]
        Kg = V.shape[2]
        off = (9 - (Kg + 2)) // 2
        Vf = V.reshape(256, -1)
        for u in range(3):
            for v in range(3):
                K9[:, :, off + u:off + u + Kg, off + v:off + v + Kg] += (
                    Wg[:, :, u, v] @ Vf).reshape(256, 256, Kg, Kg)
    return K9


def _build_deltas(inp):
    """Border strip delta kernels (exact fix for the composed conv's border:
    the two-stage form zero-clips the intermediate at ring -1/64)."""
    wcat = np.asarray(inp["conv1x1_w"], np.float32)
    Dt = np.zeros((256, 256, 3, 9), np.float32)
    Db = np.zeros((256, 256, 3, 9), np.float32)
    Dl = np.zeros((256, 256, 9, 3), np.float32)
    Dr = np.zeros((256, 256, 9, 3), np.float32)
    Cc = {k: np.zeros((256, 256, 3, 3), np.float32) for k in ("tl", "tr", "bl", "br")}
    for g, wname in ((2, "conv2_w"), (3, "conv3_w"), (4, "conv4_w")):
        V = np.asarray(inp[wname], np.float32)
        Wg = wcat[:, g * 256:(g + 1) * 256]
        h = (V.shape[2] - 1) // 2
        for u in range(3):
            for dy in range(h):
                for kx in range(2 * h + 1):
                    dx = (u - 1) + (kx - h)
                    Dt[:, :, dy, dx + 4] += Wg[:, :, 0, u] @ V[:, :, dy + h + 1, kx]
                    Db[:, :, dy, dx + 4] += Wg[:, :, 2, u] @ V[:, :, h - 1 - dy, kx]
        for t in range(3):
            for ky in range(2 * h + 1):
                dy = (t - 1) + (ky - h)
                for dxc in range(h):
                    Dl[:, :, dy + 4, dxc] += Wg[:, :, t, 0] @ V[:, :, ky, dxc + h + 1]
                    Dr[:, :, dy + 4, dxc] += Wg[:, :, t, 2] @ V[:, :, ky, h - 1 - dxc]
        for dy in range(h):
            for dx in range(h):
                Cc["tl"][:, :, dy, dx] += Wg[:, :, 0, 0] @ V[:, :, dy + h + 1, dx + h + 1]
                Cc["tr"][:, :, dy, dx] += Wg[:, :, 0, 2] @ V[:, :, dy + h + 1, h - 1 - dx]
                Cc["bl"][:, :, dy, dx] += Wg[:, :, 2, 0] @ V[:, :, h - 1 - dy, dx + h + 1]
                Cc["br"][:, :, dy, dx] += Wg[:, :, 2, 2] @ V[:, :, h - 1 - dy, h - 1 - dx]
    # corner kernels add back the double-subtracted corner ring terms -> negate
    return dict(Dt=Dt, Db=Db, Dl=Dl, Dr=Dr,
                tl=-Cc["tl"], tr=-Cc["tr"], bl=-Cc["bl"], br=-Cc["br"])


def _prep_inputs(inp):
    """Full problem inputs -> dict of packed host arrays (shared by cores)."""
    d = {}
    # reduce conv: fold BN, duplicate output channels to fill 128 partitions
    sc = np.asarray(inp["bn_g"], np.float32) / np.sqrt(
        np.asarray(inp["bn_var"], np.float32) + EPS
    )
    w_red = np.asarray(inp["reduce_w"], np.float32)[:, :, 0, 0] * sc[:, None]  # [64,256]
    b_red = (
        np.asarray(inp["reduce_b"], np.float32) - np.asarray(inp["bn_mean"], np.float32)
    ) * sc + np.asarray(inp["bn_b"], np.float32)
    w_red2 = np.concatenate([w_red, w_red], axis=0)  # [128, 256]
    d["wred"] = _f16(_pack_conv(w_red2[:, :, None, None]))  # [2,1,128,128]
    d["bred"] = np.concatenate([b_red, b_red])[:, None].astype(np.float32)  # [128,1]

    # composed 9x9 (input is M = 2*qkv0 -> fold 0.5) + pool 3x3 group
    K9 = _build_composed(inp)
    wcat = np.asarray(inp["conv1x1_w"], np.float32)
    comb = _pack_conv5(K9, 0.5)                      # [2,81,2,128,128]
    pool = _pack_conv5(wcat[:, 5 * 256:6 * 256])     # [2, 9,2,128,128]
    d["wcomb"] = _f16(np.concatenate([comb, pool], axis=1))  # [2,90,2,128,128]

    # border correction lhsT stream (0.5 folded: reads M off the q0 grid)
    arrs = _build_deltas(inp)
    tiles = []
    for e in CORR_META:
        D = arrs[e["a"]][:, :, e["i"], e["j"]] * 0.5  # [o, i]
        t = D.transpose(1, 0).reshape(2, 128, 2, 128).transpose(0, 2, 1, 3)
        tiles.append(t)
    d["wcorr"] = _f16(np.stack(tiles))  # [144, 2, 2, 128, 128]

    d["wch"] = _f16(_pack_conv5(inp["channel_conv_w"]))  # [2,9,2,128,128]
    wc = np.asarray(inp["conv_w"], np.float32)[0, :, 0, 0]  # [256]
    d["wcont16"] = _f16(wc.reshape(2, 128, 1))  # [2,128,1]
    d["wproj"] = _f16(_pack_conv(inp["proj_w"]))  # [2,1,128,256]
    d["wct1"] = _pack_conv(inp["ct1_w"]).astype(np.float32)  # [2,1,128,32]
    d["ct1b"] = np.asarray(inp["ct1_b"], np.float32)[:, None]  # [32,1]
    d["wct2"] = _pack_conv(inp["ct2_w"]).astype(np.float32)  # [1,1,32,256]
    d["ct2b"] = np.asarray(inp["ct2_b"], np.float32).reshape(2, 128, 1)
    d["ident"] = np.eye(128, dtype=np.float16)
    d["lng"] = np.asarray(inp["ln_g"], np.float32)[:, None]
    d["lnb"] = np.asarray(inp["ln_b"], np.float32)[:, None]
    return d


# ---------------------------------------------------------------------------
# kernel body
# ---------------------------------------------------------------------------
def _emit(nc, tc, ap):
    ctx = ExitStack()
    consts = ctx.enter_context(tc.tile_pool(name="consts", bufs=1))
    acts = ctx.enter_context(tc.tile_pool(name="acts", bufs=1))
    wring = ctx.enter_context(tc.tile_pool(name="wring", bufs=1))
    work = ctx.enter_context(tc.tile_pool(name="work", bufs=1))
    psum = ctx.enter_context(tc.tile_pool(name="psum", bufs=8, space="PSUM"))

    def cst(name, shape, dtype, src):
        t = consts.tile(shape, dtype, tag=name, name=name)
        nc.sync.dma_start(out=t, in_=src)
        return t

    wred = [cst(f"wred{k}", [128, 128], F16, ap["wred"][k, 0]) for k in range(2)]
    bred = cst("bred", [128, 1], F32, ap["bred"])
    wcont16 = [cst(f"wcont16{k}", [128, 1], F16, ap["wcont16"][k]) for k in range(2)]
    wproj = [cst(f"wproj{k}", [128, 256], F16, ap["wproj"][k, 0]) for k in range(2)]
    wct1 = [cst(f"wct1{k}", [128, 32], F32, ap["wct1"][k, 0]) for k in range(2)]
    wct2 = cst("wct2", [32, 256], F32, ap["wct2"][0, 0])
    ct1b = cst("ct1b", [32, 1], F32, ap["ct1b"])
    ct2b = [cst(f"ct2b{k}", [128, 1], F32, ap["ct2b"][k]) for k in range(2)]
    ident = cst("ident", [128, 128], F16, ap["ident"])
    lng = cst("lng", [32, 1], F32, ap["lng"])
    lnb = cst("lnb", [32, 1], F32, ap["lnb"])

    sigma = consts.tile([128, 1], F32, tag="sigma", name="sigma")
    nc.vector.memset(sigma[0:64, :], 1.0)
    nc.vector.memset(sigma[64:128, :], -1.0)
    epsv = consts.tile([32, 1], F32, tag="epsv", name="epsv")
    nc.vector.memset(epsv, EPS)
    onesb = consts.tile([1, 128], BF16, tag="onesb", name="onesb")
    nc.vector.memset(onesb, 1.0)
    onesf = consts.tile([32, 1], F32, tag="onesf", name="onesf")
    nc.vector.memset(onesf, 1.0)
    onesf2 = consts.tile([1, 32], F32, tag="onesf2", name="onesf2")
    nc.vector.memset(onesf2, 1.0)

    # ---- activation buffers (f16).  q0/pb are per-image (phase-1 of image
    # i+1 overlaps image i's conv pass); qkv is shared (WAR deps order the
    # next image's drains behind the previous image's tail reads).
    def padbuf(name, hw):
        return acts.tile([128, hw, hw], F16, tag=name, name=name)

    # q0 holds both images in one tile per input-channel half so the border
    # correction can matmul both images' strips with a single lhsT load.
    q0k = [
        acts.tile([128, BPC, 72, 72], F16, tag=f"q0_{k}", name=f"q0_{k}")
        for k in range(2)
    ]
    q0 = [[q0k[k][:, im] for k in range(2)] for im in range(BPC)]
    pb = [[padbuf(f"p_{im}_{k}", 66) for k in range(2)] for im in range(BPC)]
    qkv_t = [
        [
            acts.tile([128, 4480], F16, tag=f"qkv_{im}_{k}", name=f"qkv_{im}_{k}")
            for k in range(2)
        ]
        for im in range(BPC)
    ]
    qkv = [
        [t[:, 0:4356].rearrange("p (a b) -> p a b", b=66) for t in qkv_t[im]]
        for im in range(BPC)
    ]
    for im in range(BPC):
        for t in qkv_t[im]:
            nc.gpsimd.memset(t[:, 4356:4480], 0.0)

    # zero the halo borders once (interior is fully overwritten per image)
    for t in q0k:
        nc.gpsimd.memset(t[:, :, 0:4, :], 0.0)
        nc.gpsimd.memset(t[:, :, 68:72, :], 0.0)
        nc.gpsimd.memset(t[:, :, 4:68, 0:4], 0.0)
        nc.gpsimd.memset(t[:, :, 4:68, 68:72], 0.0)
    for t in [t_ for im in range(BPC) for t_ in pb[im]] + [
        t_ for im in range(BPC) for t_ in qkv[im]
    ]:
        nc.gpsimd.memset(t[:, 0:1, :], 0.0)
        nc.gpsimd.memset(t[:, 65:66, :], 0.0)
        nc.gpsimd.memset(t[:, 1:65, 0:1], 0.0)
        nc.gpsimd.memset(t[:, 1:65, 65:66], 0.0)

    # ---- phase 1: stream x -> reduce conv+ReLU -> DWT -> M;  maxpool -> p
    class Ph1:
        def __init__(self, img):
            self.img = img
            self.xts = []
            self.sc = 0

        def emit_dmas(self, scs=range(16)):
            for sc_ in scs:
                pair = []
                for k in range(2):
                    xt = work.tile(
                        [128, 8, 128], F16, tag=f"x{k}", bufs=8, name=f"xt{k}"
                    )
                    src = ap["x"][
                        self.img, k * 128 : (k + 1) * 128, sc_ * 8 : sc_ * 8 + 8, :
                    ]
                    nc.sync.dma_start(out=xt, in_=src)
                    pair.append(xt)
                self.xts.append(pair)

        def step(self, n=1):
            for _ in range(n):
                if self.sc < 16:
                    self._sc(self.sc)
                    self.sc += 1

        def _sc(self, sc_):
            xts = self.xts[sc_]
            orow = sc_ * 4  # 8 input rows -> 4 output rows
            rch = work.tile([128, 8, 128], F16, tag="rch", bufs=2, name="rch")
            for sub in range(2):
                ps = psum.tile([128, 4, 128], F32, tag="ps", name="ps_r")
                for k in range(2):
                    nc.tensor.matmul(
                        ps, wred[k], xts[k][:, sub * 4 : sub * 4 + 4, :],
                        start=(k == 0), stop=(k == 1),
                    )
                nc.scalar.activation(
                    out=rch[:, sub * 4 : sub * 4 + 4, :], in_=ps, func=AF.Relu,
                    bias=bred, scale=1.0,
                )
            rv = rch.rearrange("p (a two) (c cp) -> p a two c cp", two=2, cp=2)
            a_, b_ = rv[:, :, 0, :, 0], rv[:, :, 0, :, 1]
            c_, d_ = rv[:, :, 1, :, 0], rv[:, :, 1, :, 1]
            u = work.tile([128, 4, 64], F32, tag="u", bufs=2, name="u")
            v = work.tile([128, 4, 64], F32, tag="v", bufs=2, name="v")
            s_ = work.tile([128, 4, 64], F32, tag="s", bufs=2, name="s_")
            t_ = work.tile([128, 4, 64], F32, tag="t", bufs=2, name="t_")
            nc.vector.tensor_add(u, a_, b_)
            nc.vector.tensor_add(v, c_, d_)
            nc.vector.tensor_sub(s_, a_, b_)
            nc.vector.tensor_sub(t_, c_, d_)
            sv = work.tile([128, 4, 64], F32, tag="sv", bufs=2, name="sv")
            st = work.tile([128, 4, 64], F32, tag="st", bufs=2, name="st")
            # sigma-scale on the Scalar engine to unload DVE
            nc.scalar.activation(out=sv, in_=v, func=AF.Copy, scale=sigma)
            nc.scalar.activation(out=st, in_=t_, func=AF.Copy, scale=sigma)
            myq0 = q0[self.img]
            nc.vector.tensor_add(myq0[0][:, 4 + orow : 8 + orow, 4:68], u, sv)
            nc.vector.tensor_add(myq0[1][:, 4 + orow : 8 + orow, 4:68], s_, st)
            for k in range(2):
                xv = xts[k].rearrange("p (a two) (c cp) -> p a two c cp", two=2, cp=2)
                xa, xb = xv[:, :, 0, :, 0], xv[:, :, 0, :, 1]
                xc, xd = xv[:, :, 1, :, 0], xv[:, :, 1, :, 1]
                m1 = work.tile([128, 4, 64], F16, tag="m1", bufs=1, name="m1")
                m2 = work.tile([128, 4, 64], F16, tag="m2", bufs=1, name="m2")
                nc.vector.tensor_max(m1, xa, xb)
                nc.vector.tensor_max(m2, xc, xd)
                nc.vector.tensor_max(pb[self.img][k][:, 1 + orow : 5 + orow, 1:65], m1, m2)

    # ---- main pass: accumulate all 90 taps x 2 ktiles into 8 psum banks
    # (2 mt x 4 chunks of 8 rows) per spatial half.
    def conv_pass(img, boundary_hooks, inloop=None):
        g0t = q0[img]
        pbt = pb[img]
        for qh in range(2):
            pss = [
                [psum.tile([128, 8, 64], F32, tag="ps", name="ps_c") for _ in range(4)]
                for _ in range(2)
            ]
            n = 0
            for ik in range(2):
                for bq in range(15):
                    if inloop and qh == 0:
                        fn = inloop.pop(ik * 15 + bq, None)
                        if fn:
                            fn()
                    wt = wring.tile([128, 1536], F16, tag="wtap6", bufs=3, name="wt")
                    nc.sync.dma_start(
                        out=wt,
                        in_=ap["wcomb"][ik, bq * 6 : bq * 6 + 6].rearrange(
                            "t m p c -> p (t m) c"
                        ),
                    )
                    for ti in range(6):
                        tp = bq * 6 + ti
                        for mt in range(2):
                            lhsT = wt[:, (ti * 2 + mt) * 128 : (ti * 2 + mt + 1) * 128]
                            for ci in range(4):
                                r0 = qh * 32 + ci * 8
                                if tp < 81:
                                    ky, kx = tp // 9, tp % 9
                                    rhs = g0t[ik][:, ky + r0 : ky + r0 + 8, kx : kx + 64]
                                else:
                                    tpp = tp - 81
                                    ky, kx = tpp // 3, tpp % 3
                                    rhs = pbt[ik][:, ky + r0 : ky + r0 + 8, kx : kx + 64]
                                nc.tensor.matmul(
                                    pss[mt][ci], lhsT, rhs,
                                    start=(n == 0), stop=(n == 179),
                                )
                        n += 1
            for ci in range(4):
                r0 = qh * 32 + ci * 8
                nc.vector.tensor_copy(qkv[img][0][:, 1 + r0 : 9 + r0, 1:65], pss[0][ci])
                nc.scalar.copy(qkv[img][1][:, 1 + r0 : 9 + r0, 1:65], pss[1][ci])
            boundary_hooks[qh]()

    # ---- exact border correction: 4 strip convs + corner add-backs into one
    # psum bank per image (both images share each streamed weight tile -> the
    # 18MB correction stream is paid once), then subtract from qkv borders.
    corr_sbs = [None, None]

    def corr_accum_both():
        # bank1: top/bottom strips, bank2: left/right (+corner pts); each
        # matmul covers BOTH images via the combined q0 layout (free = 2x64).
        bank1 = psum.tile([128, BPC, 4, 64], F32, tag="ps", name="corr_ps1")
        bank2 = psum.tile([128, BPC, 4, 64], F32, tag="ps", name="corr_ps2")
        for bi in range(36):
          wt = wring.tile([128, 2048], F16, tag="wcorrw4", bufs=2, name="wc")
          nc.sync.dma_start(
              out=wt,
              in_=ap["wcorr"][bi * 4 : bi * 4 + 4].rearrange(
                  "t k m p c -> p (t k m) c"
              ),
          )
          for ti in range(4):
            idx = bi * 4 + ti
            e = CORR_META[idx]
            s = e["s"]
            if idx == CORR_FIRST[s]:
                # chain opener: start=True matmul over a guaranteed-zero region
                # of the padded grid (rows/cols 0..3 are zero), so every real
                # tap accumulates (start=True contributions land unreliably on
                # sub-bank regions).
                for mt in range(2):
                    if e["kind"] == "row":
                        z_rhs = q0k[0][:, :, 0:1, 0:64]
                        z_dst = bank1[:, :, s * 2 + mt : s * 2 + mt + 1, :]
                    else:
                        z_rhs = q0k[0][:, :, 0:64, 0:1]
                        sl = (s - 2) * 2 + mt
                        z_dst = bank2[:, :, sl : sl + 1, :].rearrange(
                            "p a b c -> p a c b"
                        )
                    nc.tensor.matmul(
                        z_dst, wt[:, (ti * 4 + mt) * 128 : (ti * 4 + mt + 1) * 128],
                        z_rhs, start=True, stop=False,
                    )
            for ik in range(2):
                if e["kind"] == "row":
                    rhs = q0k[ik][:, :, e["row"] : e["row"] + 1, e["c0"] : e["c0"] + 64]
                    dsts = [bank1[:, :, s * 2 + mt : s * 2 + mt + 1, :]
                            for mt in range(2)]
                elif e["kind"] == "col":
                    rhs = q0k[ik][:, :, e["r0"] : e["r0"] + 64, e["col"] : e["col"] + 1]
                    dsts = [bank2[:, :, (s - 2) * 2 + mt : (s - 2) * 2 + mt + 1, :]
                            .rearrange("p a b c -> p a c b") for mt in range(2)]
                else:
                    rhs = q0k[ik][:, :, e["row"] : e["row"] + 1, e["col"] : e["col"] + 1]
                    dsts = [bank2[:, :, (s - 2) * 2 + mt : (s - 2) * 2 + mt + 1,
                                  e["d0"] : e["d0"] + 1] for mt in range(2)]
                for mt in range(2):
                    j = ti * 4 + ik * 2 + mt
                    nc.tensor.matmul(
                        dsts[mt],
                        wt[:, j * 128 : (j + 1) * 128],
                        rhs,
                        start=False,
                        stop=(idx == CORR_LAST[s] and ik == 1),
                    )
        for bn, bank in ((0, bank1), (1, bank2)):
            sb = work.tile([128, BPC, 4, 64], F32, tag=f"corr{bn}", name=f"corr_sb{bn}")
            nc.scalar.copy(sb, bank)
            corr_sbs[bn] = sb

    def corr_apply(img):
        sb1 = corr_sbs[0][:, img]
        sb2 = corr_sbs[1][:, img]
        for mt in range(2):
            qv = qkv[img][mt]
            nc.vector.tensor_sub(
                qv[:, 1:2, 1:65], qv[:, 1:2, 1:65], sb1[:, 0 + mt : 1 + mt, :]
            )
            nc.vector.tensor_sub(
                qv[:, 64:65, 1:65], qv[:, 64:65, 1:65], sb1[:, 2 + mt : 3 + mt, :]
            )
            nc.vector.tensor_sub(
                qv[:, 1:65, 1:2], qv[:, 1:65, 1:2],
                sb2[:, 0 + mt : 1 + mt, :].rearrange("p a b -> p b a"),
            )
            nc.vector.tensor_sub(
                qv[:, 1:65, 64:65], qv[:, 1:65, 64:65],
                sb2[:, 2 + mt : 3 + mt, :].rearrange("p a b -> p b a"),
            )

    # ---- content logits + softmax numerator/denominator on the (corrected)
    # qkv tiles; e-maps bounced to DRAM scratch for the shifted gathers.
    def content_pass(img):
        for ci in range(8):
            r0 = ci * 8
            cp = psum.tile([1, 8, 64], F32, tag="ps", name="cp")
            for k in range(2):
                nc.tensor.matmul(
                    cp, wcont16[k], qkv[img][k][:, 1 + r0 : 9 + r0, 1:65],
                    start=(k == 0), stop=(k == 1),
                )
            nc.scalar.activation(
                out=e66i[:, 1 + r0 : 9 + r0, 1:65], in_=cp, func=AF.Exp,
                bias=0.0, scale=1.0, accum_out=denc[:, ci : ci + 1],
            )
        # bounce the whole e-grid once, then 9 full-width shifted gathers
        lo = 134 + 66
        nc.sync.dma_start(
            out=ap["e_scr"][:, lo : lo + 64 * 66], in_=e66_sb[:, lo : lo + 64 * 66]
        )
        for k9 in range(9):
            ky, kx = k9 // 3, k9 % 3
            off = 201 - 66 * ky - kx
            nc.sync.dma_start(
                out=aT[:, k9, :],
                in_=ap["e_scr"][:, off : off + 4480].rearrange(
                    "o (p t) -> (o p) t", t=35
                ),
            )

    # ---- attention tail part A: softmax denominator, then the attention
    # pooling s[i,tap] = sum_n e_n * qkv[i, n+tap] entirely on the PE:
    # transpose qkv spatial->partition tile by tile, contract against 9
    # shifted (unnormalized) e-maps gathered by strided DMA, normalize by
    # 1/den when casting s to f16, transpose s back to channel-major.
    def tail_a(img):
        dent = work.tile([1, 1], F32, tag="dent", name="dent")
        nc.vector.tensor_reduce(
            dent, denc, axis=mybir.AxisListType.X, op=ALU.add
        )
        rden = work.tile([1, 1], F32, tag="rden", name="rden")
        nc.vector.reciprocal(rden, dent)
        rden_bf = work.tile([1, 1], BF16, tag="rden_bf", name="rden_bf")
        nc.vector.tensor_copy(rden_bf, rden)
        rd_ps = psum.tile([128, 1], F32, tag="ps", name="rd_ps")
        nc.tensor.matmul(rd_ps, onesb, rden_bf, start=True, stop=True)
        rdsb = work.tile([128, 1], F32, tag="rdsb", name="rdsb")
        nc.vector.tensor_copy(rdsb, rd_ps)
        # 9 shifted views of the padded e-grid, spatial-on-partitions with
        # m = p*35 + t so every partition reads one contiguous 70B run.
        # (SBUF partition dims can't alias free-dim strides, so bounce the
        # flat e-grid through DRAM scratch and gather from there.)
        qv35 = [
            qkv_t[img][ik].rearrange("p (m t) -> p t m", t=35) for ik in range(2)
        ]
        smm = psum.tile([9, 256], F32, tag="ps", name="smm")
        for t in range(35):
            qpT = work.tile([128, 2, 128], BF16, tag="qpT", bufs=4, name="qpT")
            for ik in range(2):
                tp_ps = psum.tile([128, 128], F16, tag="ps", name="tp_ps")
                nc.tensor.transpose(tp_ps, qv35[ik][:, t, :], ident)
                if ik == 0:
                    nc.scalar.copy(qpT[:, ik, :], tp_ps)
                else:
                    nc.vector.tensor_copy(qpT[:, ik, :], tp_ps)
            nc.tensor.matmul(
                smm, aT[:, :, t], qpT.rearrange("p a b -> p (a b)"),
                start=(t == 0), stop=(t == 34),
            )
        s_sb = work.tile([9, 256], F16, tag="s_sb", name="s_sb")
        nc.scalar.activation(
            out=s_sb, in_=smm, func=AF.Copy, scale=rdsb[0:9, :]
        )
        for ik in range(2):
            st_ps = psum.tile([128, 9], F16, tag="ps", name="st_ps")
            nc.tensor.transpose(
                st_ps, s_sb[:, ik * 128 : (ik + 1) * 128], ident[0:9, 0:9]
            )
            nc.scalar.copy(s16[ik], st_ps)

    # ---- attention tail part B: pooled matvec, channel transform, proj conv
    def tail_b(img):
        pooled_ps = [
            psum.tile([128, 1], F32, tag="ps", name=f"pool_ps{mt}") for mt in range(2)
        ]
        for ik in range(2):
            wt = wring.tile([128, 2304], F16, tag="wchw", bufs=1, name="wtc")
            nc.sync.dma_start(
                out=wt, in_=ap["wch"][ik].rearrange("t m p c -> p (t m) c")
            )
            for tp in range(9):
                for mt in range(2):
                    nc.tensor.matmul(
                        pooled_ps[mt],
                        wt[:, (tp * 2 + mt) * 128 : (tp * 2 + mt + 1) * 128],
                        s16[ik][:, tp : tp + 1],
                        start=(ik == 0 and tp == 0), stop=(ik == 1 and tp == 8),
                    )
        pooled = []
        for mt in range(2):
            pl = work.tile([128, 1], F32, tag=f"pool{mt}", name="pl")
            nc.vector.tensor_copy(pl, pooled_ps[mt])
            pooled.append(pl)

        # channel transform (tiny, fp32)
        t_ps = psum.tile([32, 1], F32, tag="ps", name="t_ps")
        for k in range(2):
            nc.tensor.matmul(t_ps, wct1[k], pooled[k], start=(k == 0), stop=(k == 1))
        ts2 = work.tile([32, 2], F32, tag="ts2", name="ts2")
        t_sb = ts2[:, 0:1]
        nc.vector.tensor_scalar_add(t_sb, t_ps, ct1b)
        nc.vector.tensor_mul(ts2[:, 1:2], t_sb, t_sb)
        sums_ps = psum.tile([1, 2], F32, tag="ps", name="sums_ps")
        nc.tensor.matmul(sums_ps, onesf, ts2, start=True, stop=True)
        sums_sb = work.tile([1, 2], F32, tag="sums_sb", name="sums_sb")
        nc.vector.tensor_copy(sums_sb, sums_ps)
        bc_ps = psum.tile([32, 2], F32, tag="ps", name="bc_ps")
        nc.tensor.matmul(bc_ps, onesf2, sums_sb, start=True, stop=True)
        mean = work.tile([32, 1], F32, tag="mean", name="mean")
        nc.vector.tensor_scalar_mul(mean, bc_ps[:, 0:1], 1.0 / 32)
        mv = work.tile([32, 1], F32, tag="mv", name="mv")
        nc.vector.tensor_scalar_mul(mv, bc_ps[:, 1:2], 1.0 / 32)
        m2t = work.tile([32, 1], F32, tag="m2t", name="m2t")
        nc.vector.tensor_mul(m2t, mean, mean)
        var = work.tile([32, 1], F32, tag="var", name="var")
        nc.vector.tensor_sub(var, mv, m2t)
        sd = work.tile([32, 1], F32, tag="sd", name="sd")
        nc.scalar.activation(out=sd, in_=var, func=AF.Sqrt, bias=epsv, scale=1.0)
        rsd = work.tile([32, 1], F32, tag="rsd", name="rsd")
        nc.vector.reciprocal(rsd, sd)
        dt_ = work.tile([32, 1], F32, tag="dt", name="dt_")
        nc.vector.tensor_sub(dt_, t_sb, mean)
        tn = work.tile([32, 1], F32, tag="tn", name="tn")
        nc.vector.tensor_mul(tn, dt_, rsd)
        tact = work.tile([32, 1], F32, tag="tact", name="tact")
        nc.scalar.activation(out=tact, in_=tn, func=AF.Relu, bias=lnb, scale=lng)

        projs = []
        for mt in range(2):
            cw_ps = psum.tile([128, 1], F32, tag="ps", name="cw_ps")
            nc.tensor.matmul(
                cw_ps, wct2[:, mt * 128 : (mt + 1) * 128], tact, start=True, stop=True
            )
            cw = work.tile([128, 1], F32, tag=f"cw{mt}", name="cw")
            nc.vector.tensor_scalar_add(cw, cw_ps, ct2b[mt])
            pj = work.tile([128, 256], F16, tag=f"projs{mt}", name="pj")
            nc.vector.tensor_scalar_mul(pj, wproj[mt], cw)
            projs.append(pj)

        # out = proj(qkv * cw)  (cw folded into proj weights)
        for mt in range(2):
            for ci in range(8):
                r0 = ci * 8
                po = psum.tile([128, 8, 64], F32, tag="ps", name="po")
                for k in range(2):
                    nc.tensor.matmul(
                        po,
                        projs[k][:, mt * 128 : (mt + 1) * 128],
                        qkv[img][k][:, 1 + r0 : 9 + r0, 1:65],
                        start=(k == 0), stop=(k == 1),
                    )
                ost = work.tile([128, 8, 64], F16, tag="ost", bufs=2, name="ost")
                if ci % 2 == 0:
                    nc.scalar.copy(ost, po)
                else:
                    nc.vector.tensor_copy(ost, po)
                nc.sync.dma_start(
                    out=ap["out"][img, mt * 128 : (mt + 1) * 128, r0 : r0 + 8, :],
                    in_=ost,
                )

    # shared tail tiles.  e66_sb: e embedded at offset 134 on a 66-wide,
    # zero-bordered grid (margins make all 9 shifted DMA gathers in-range).
    e66_sb = work.tile([1, 4736], BF16, tag="e66", name="e66")
    nc.gpsimd.memset(e66_sb, 0.0)
    e66i = e66_sb[:, 134 : 134 + 4488].rearrange("o (r c) -> o r c", c=66)
    denc = work.tile([1, 8], F32, tag="denc", name="denc")
    aT = work.tile([128, 9, 35], BF16, tag="aT", name="aT")
    nc.sync.dma_start(out=ap["e_scr"], in_=e66_sb)  # zero borders once
    s16 = [work.tile([128, 9], F16, tag=f"s16_{ik}", name="s16") for ik in range(2)]

    # ---- schedule ----
    ph1 = [Ph1(im) for im in range(BPC)]
    ph1[0].emit_dmas()
    ph1[0].step(16)

    def hook0():
        ph1[1].step(8)
        ph1[1].emit_dmas(range(8, 16))

    def hook1():
        ph1[1].step(8)

    def hookA():
        # pass(1) qh0/qh1 boundary: both images' border corrections (shared
        # weight stream), then image 0's whole attention prologue -- the
        # e-map bounce/gather latency and corr-stream gaps cover each other,
        # and qh1 runs while image 0's tail state settles.
        corr_accum_both()
        corr_apply(0)
        if "dbgq" in ap:
            for ik in range(2):
                nc.sync.dma_start(out=ap["dbgq"][0, ik], in_=qkv_t[0][ik][:, 0:4356])
        content_pass(0)

    conv_pass(0, [hook0, hook1],
              inloop={10: lambda: ph1[1].emit_dmas(range(0, 8))})
    conv_pass(1, [hookA, lambda: tail_a(0)])
    corr_apply(1)
    if "dbgq" in ap:
        for ik in range(2):
            nc.sync.dma_start(out=ap["dbgq"][1, ik], in_=qkv_t[1][ik][:, 0:4356])
    content_pass(1)
    tail_b(0)
    tail_a(1)
    tail_b(1)
    ctx.close()
    return None


def build():
    nc = bass.Bass("TRN2", target_bir_lowering=False, debug=False)
    shapes = {
        "x": ([BPC, C, H, W], F16),
        "wred": ([2, 1, 128, 128], F16),
        "bred": ([128, 1], F32),
        "wcomb": ([2, 90, 2, 128, 128], F16),
        "wcorr": ([144, 2, 2, 128, 128], F16),
        "wch": ([2, 9, 2, 128, 128], F16),
        "wcont16": ([2, 128, 1], F16),
        "wproj": ([2, 1, 128, 256], F16),
        "wct1": ([2, 1, 128, 32], F32),
        "ct1b": ([32, 1], F32),
        "wct2": ([1, 1, 32, 256], F32),
        "ct2b": ([2, 128, 1], F32),
        "ident": ([128, 128], F16),
        "lng": ([32, 1], F32),
        "lnb": ([32, 1], F32),
    }
    ap = {
        k: nc.dram_tensor(k, shp, dt, kind="ExternalInput").ap()
        for k, (shp, dt) in shapes.items()
    }
    ap["out"] = nc.dram_tensor("out", [BPC, C, H2, W2], F16, kind="ExternalOutput").ap()
    ap["e_scr"] = nc.dram_tensor("e_scr", [1, 4736], BF16, kind="Internal").ap()
    if os.environ.get("DWT_DBG"):
        ap["dbgq"] = nc.dram_tensor(
            "dbgq", [BPC, 2, 128, 4356], F16, kind="ExternalOutput"
        ).ap()
    with tile.TileContext(nc) as tc:
        _emit(nc, tc, ap)
    return nc


_CACHED_NC = {}


def _install_trace_hook():
    """The image's antenv lacks axon_hooks; shim it and register the boot's
    ctypes NTFF hook so trace=True works.  Also neutralize the S3 artifact
    upload (no bucket access here)."""
    import types
    import antenv

    if "antenv.axon_hooks" not in sys.modules:
        mod = types.ModuleType("antenv.axon_hooks")
        mod._hook = None
        def set_axon_ntff_profile_hook(h):
            mod._hook = h
        def get_axon_ntff_profile_hook():
            return mod._hook
        mod.set_axon_ntff_profile_hook = set_axon_ntff_profile_hook
        mod.get_axon_ntff_profile_hook = get_axon_ntff_profile_hook
        sys.modules["antenv.axon_hooks"] = mod
        antenv.axon_hooks = mod
        from trn_agent_boot.trn_boot import _ntff_profile_via_ctypes
        mod.set_axon_ntff_profile_hook(
            _ntff_profile_via_ctypes("/opt/axon/libaxon_pjrt.so")
        )
        bass_utils.upload_artifacts = lambda tmpdir: tmpdir


def run(inputs, debug=False, trace=False):
    if trace:
        _install_trace_hook()
    if "nc" not in _CACHED_NC:
        _CACHED_NC["nc"] = build()
    nc = _CACHED_NC["nc"]
    d = _prep_inputs(inputs)
    x_f16 = np.asarray(inputs["x"], np.float32).astype(np.float16)
    in_maps = []
    for c in range(N_CORES):
        m = dict(d)
        m["x"] = np.ascontiguousarray(x_f16[c * BPC : (c + 1) * BPC])
        in_maps.append(m)
    res = bass_utils.run_bass_kernel_spmd(
        nc, in_maps, core_ids=list(range(N_CORES)), trace=trace
    )
    out = np.concatenate(
        [res.results[c]["out"].astype(np.float32) for c in range(N_CORES)], axis=0
    )
    return out, res


def kernel(**inputs):
    out, _ = run(inputs)
    return out
